# revision 21
# baseline (speedup 1.0000x reference)
"""Sparse-attention ("Castle") Trainium2 kernel, 8-core SPMD.

Sharding: core c handles batch b = c // 4 and head pair p = c % 4
(heads 2p, 2p+1). All fp16 operands (scale pre-folded host-side).

Single dense 4-window pipeline per core: each 512-token window
interleaves QKV projection, lookahead^T / term1^T builds, the
block-sparse Su contraction chunk, softmax, and the output projection
so the tensor engine always has independent matmuls in flight (HAM
stays at full clock as much as possible).

Su interior is collapsed via a cumulative rank-64 matrix W_c^T =
sum_{j<512c} sigma_kj vu_j^T (one K=64 matmul per k-tile instead of
up to 12 K=128 matmuls); only the 4 boundary j-tiles per chunk use
the explicit term1^T rows, so term1^T shrinks to one narrow row per
j-tile. Post-processing uses only the {tanh, exp, copy} activation
table (no table thrash): -silu(Su) comes from one AFFINE_MUL_REDUCE
custom-DVE op, sigma fixups run on gpsimd, and softmax normalization
uses reciprocal_approx_fast + gpsimd partition broadcast. The pots
(attn @ vc) matmuls are software-pipelined two pair-heads behind
their exp producers. Host sums the 4 fp16 partials per batch.
"""
import sys
sys.path.insert(0, "/opt/trn_rl_repo")
import numpy as np

B, N, D = 2, 2048, 1024
HEADS, DH = 8, 64
NT = N // 128          # 16 row tiles
SCALE = DH ** -0.5
VCW = 66               # stride of one vcA block (64 vc cols + 1 ones + pad)

# const pack layout (columns of cst tile)
C_MD = 0        # [128,128] diag mask: col >= row
C_ML = 128      # [128,128] strictly-lower mask: col < row
C_ID = 256      # [128,128] identity
C_WO = 384      # [64,2048] W_out slices, cols h*1024+d (partitions 0-63)
C_ON = 2432     # [128,8] ones
C_W = 2440

_STATE = {}


def _build_nc():
    import concourse.bacc as bacc
    import concourse.mybir as mybir
    from concourse import tile
    from concourse.dve_ops import AFFINE_MUL_REDUCE

    F32 = mybir.dt.float32
    F16 = mybir.dt.float16
    AF = mybir.ActivationFunctionType

    nc = bacc.Bacc("TRN2", target_bir_lowering=False, debug=False)

    xTp = nc.dram_tensor("xTp", [128, 4 * 4096], F16, kind="ExternalInput")
    wqp = nc.dram_tensor("wqp", [128, 8 * 768], F16, kind="ExternalInput")
    cstd = nc.dram_tensor("cstd", [128, C_W], F16, kind="ExternalInput")
    y = nc.dram_tensor("y", [N, D], F16, kind="ExternalOutput")

    with tile.TileContext(nc) as tc:
        with tc.tile_pool(name="const", bufs=1) as cstp, \
             tc.tile_pool(name="qkv", bufs=1) as qkvp, \
             tc.tile_pool(name="vca", bufs=1) as vcap, \
             tc.tile_pool(name="otn", bufs=1) as otnp, \
             tc.tile_pool(name="t1p", bufs=1) as t1p, \
             tc.tile_pool(name="ltp", bufs=1) as ltp, \
             tc.tile_pool(name="wk", bufs=3) as wk, \
             tc.tile_pool(name="xw", bufs=1) as xw, \
             tc.tile_pool(name="psC", bufs=2, space="PSUM") as psC:

            cst = cstp.tile([128, C_W], F16, tag="cst")
            mD = cst[:, C_MD:C_MD + 128]
            mL = cst[:, C_ML:C_ML + 128]
            idn = cst[:, C_ID:C_ID + 128]
            wo = cst[0:64, C_WO:C_WO + 2048]
            ones = cst[:, C_ON:C_ON + 1]

            qk = [qkvp.tile([128, N], F16, tag=f"qk{j}", name=f"qk{j}")
                  for j in range(5)]
            vca = [vcap.tile([128, VCW * NT], F16, tag=f"vca{h}",
                             name=f"vca{h}") for h in range(2)]
            vua = [vcap.tile([128, 64 * NT], F16, tag=f"vua{h}",
                             name=f"vua{h}") for h in range(2)]
            otn = [otnp.tile([64, N], F16, tag=f"otn{h}", name=f"otn{h}")
                   for h in range(2)]
            # cumulative interior weights W_c^T, rows h*64+dh, cols k
            wt = otnp.tile([128, N], F16, tag="wt", name="wt")
            nc.vector.memset(wt[:], 0.0)

            vcT = xw.tile([128, N], F16, tag="vcT", name="vcT")
            wq = xw.tile([128, 8 * 768], F16, tag="wq", name="wq")
            nc.sync.dma_start(out=wq[:], in_=wqp.ap())
            xt = []
            for nch in range(4):
                xti = xw.tile([128, 4096], F16, tag="xti", name=f"x{nch}",
                              bufs=2)
                nc.sync.dma_start(
                    out=xti[:],
                    in_=xTp.ap()[:, nch * 4096:(nch + 1) * 4096],
                )
                xt.append(xti)
            nc.sync.dma_start(out=cst[:], in_=cstd.ap())

            t1t = {}
            ltt = {}
            tg = [0]

            def copy_any(dst, src):
                if tg[0] % 2 == 0:
                    nc.vector.tensor_copy(dst, src)
                else:
                    nc.scalar.activation(dst, src, AF.Copy)
                tg[0] += 1

            def ensure_tiles(jt):
                if (0, jt) in t1t:
                    return
                r = jt % 4
                for h in range(2):
                    t1t[(h, jt)] = t1p.tile(
                        [128, 512 - 128 * r], F16, tag=f"t1r{r}_{h}",
                        name=f"t1_{h}_{jt}", bufs=2)
                    ltt[(h, jt)] = ltp.tile(
                        [128, 128 * (jt + 1)], F16, tag=f"lt_{h}_{jt}",
                        name=f"lt_{h}_{jt}")

            def emit_proj(nch):
                nsl = slice(nch * 512, nch * 512 + 512)
                for j in range(6):
                    pp = psC.tile([128, 512], F32, tag="mk", name="pp", bufs=2)
                    for dtile in range(8):
                        nc.tensor.matmul(
                            pp[:],
                            wq[:, dtile * 768 + j * 128:
                               dtile * 768 + (j + 1) * 128],
                            xt[nch][:, dtile * 512:(dtile + 1) * 512],
                            start=(dtile == 0),
                            stop=(dtile == 7),
                        )
                    dst = vcT if j == 5 else qk[j]
                    nc.scalar.activation(dst[:, nsl], pp[:], AF.Copy)

            def emit_transposes(nch):
                for kt in range(4 * nch, 4 * nch + 4):
                    pt = psC.tile([128, 128], F16, tag="mk", name="pt", bufs=2)
                    nc.tensor.transpose(
                        pt[:], vcT[:, kt * 128:(kt + 1) * 128], idn)
                    for h in range(2):
                        nc.vector.tensor_copy(
                            vca[h][:, kt * VCW:kt * VCW + 64],
                            pt[:, h * 64:(h + 1) * 64],
                        )
                        nc.vector.tensor_copy(
                            vca[h][:, kt * VCW + 64:kt * VCW + 65], ones
                        )
                    pu = psC.tile([128, 128], F16, tag="mk", name="pu", bufs=2)
                    nc.tensor.transpose(
                        pu[:], qk[2][:, kt * 128:(kt + 1) * 128], idn)
                    for h in range(2):
                        nc.vector.tensor_copy(
                            vua[h][:, kt * 64:(kt + 1) * 64],
                            pu[:, h * 64:(h + 1) * 64],
                        )

            def emit_t1_row(jt):
                i0 = 128 * jt
                w = 512 - 128 * (jt % 4)
                pss = []
                for h in range(2):
                    vuT = qk[2][h * 64:h * 64 + 64, :]
                    qcT = qk[3][h * 64:h * 64 + 64, :]
                    ps = psC.tile([128, 512], F32, tag="mk",
                                  name="mkp", bufs=2)
                    nc.tensor.matmul(
                        ps[:, 0:w],
                        vuT[:, jt * 128:(jt + 1) * 128],
                        qcT[:, i0:i0 + w],
                        start=True, stop=True,
                    )
                    pss.append(ps)
                for h in range(2):
                    t1 = t1t[(h, jt)]
                    ps = pss[h]
                    nc.vector.tensor_mul(t1[:, 0:128], ps[:, 0:128], mD)
                    if w > 128:
                        copy_any(t1[:, 128:w], ps[:, 128:w])

            def emit_lt_row(jt):
                Lk = 128 * (jt + 1)
                for k0 in range(0, Lk, 512):
                    w = min(512, Lk - k0)
                    pss = []
                    for h in range(2):
                        quT = qk[0][h * 64:h * 64 + 64, :]
                        kuT = qk[1][h * 64:h * 64 + 64, :]
                        ps = psC.tile([128, 512], F32, tag="mk",
                                      name="mkp2", bufs=2)
                        nc.tensor.matmul(
                            ps[:, 0:w],
                            kuT[:, jt * 128:(jt + 1) * 128],
                            quT[:, k0:k0 + w],
                            start=True, stop=True,
                        )
                        pss.append(ps)
                    last = (k0 + w == Lk)
                    for h in range(2):
                        lt = ltt[(h, jt)]
                        thL = wk.tile([128, 512], F16, tag="thL",
                                      name="thL", bufs=2)
                        nc.scalar.activation(thL[:, 0:w], pss[h][:, 0:w],
                                             AF.Tanh, scale=0.5)
                        wa = w - 128 if last else w
                        if wa > 0:
                            nc.gpsimd.tensor_scalar(
                                lt[:, k0:k0 + wa], thL[:, 0:wa], 0.5, 0.5,
                                op0=mybir.AluOpType.mult,
                                op1=mybir.AluOpType.add,
                            )
                        if last:
                            # lt_diag = (th + 1) * (0.5*mask)
                            nc.vector.scalar_tensor_tensor(
                                lt[:, Lk - 128:Lk], thL[:, w - 128:w], 1.0, mL,
                                op0=mybir.AluOpType.add,
                                op1=mybir.AluOpType.mult,
                            )

            def w_update(c):
                # add j-window [512(c-1), 512c) into W^T for k-tiles < 4c
                for h in range(2):
                    for kt in range(0, 4 * c):
                        jts = [jt for jt in range(4 * (c - 1), 4 * c)
                               if jt >= kt]
                        if not jts:
                            continue
                        pd = psC.tile([128, 128], F32, tag="mk",
                                      name="pd", bufs=2)
                        for ji, jt in enumerate(jts):
                            nc.tensor.matmul(
                                pd[h * 64:h * 64 + 64, :],
                                vua[h][:, jt * 64:(jt + 1) * 64],
                                ltt[(h, jt)][:, kt * 128:(kt + 1) * 128],
                                start=(ji == 0), stop=(ji == len(jts) - 1),
                            )
                        nc.vector.tensor_add(
                            wt[h * 64:h * 64 + 64, kt * 128:(kt + 1) * 128],
                            wt[h * 64:h * 64 + 64, kt * 128:(kt + 1) * 128],
                            pd[h * 64:h * 64 + 64, :],
                        )

            def chunks_both(c):
                csl = slice(512 * c, 512 * c + 512)
                tmax = 4 * c + 3
                pots = {}
                for h in range(2):
                    pots[h] = psC.tile([65, 512], F32, tag=f"ot{h}",
                                       name=f"pot{h}", bufs=1)

                def emit_psu_pair(tp):
                    psus = [psC.tile([128, 1024], F32, tag="su",
                                     name="psu", bufs=2) for _ in range(2)]
                    for half in range(2):
                        t = tp + half
                        hof = 512 * half
                        started = [False, False]
                        if t < 4 * c:
                            for h in range(2):
                                qcT = qk[3][h * 64:h * 64 + 64, :]
                                nc.tensor.matmul(
                                    psus[h][:, hof:hof + 512],
                                    wt[h * 64:h * 64 + 64,
                                       t * 128:(t + 1) * 128],
                                    qcT[:, csl],
                                    start=True, stop=False,
                                )
                                started[h] = True
                        for jt in range(max(t, 4 * c), tmax + 1):
                            s_loc = 512 * c - 128 * jt
                            for h in range(2):
                                if s_loc >= 0:
                                    nc.tensor.matmul(
                                        psus[h][:, hof:hof + 512],
                                        ltt[(h, jt)][:, 128 * t:128 * t + 128],
                                        t1t[(h, jt)][:, s_loc:s_loc + 512],
                                        start=not started[h],
                                        stop=(jt == tmax),
                                    )
                                else:
                                    nc.tensor.matmul(
                                        psus[h][:, hof - s_loc:hof + 512],
                                        ltt[(h, jt)][:, 128 * t:128 * t + 128],
                                        t1t[(h, jt)][:, 0:512 + s_loc],
                                        start=not started[h],
                                        stop=(jt == tmax),
                                    )
                                started[h] = True
                    return psus

                def emit_psc(tp, h):
                    qcT = qk[3][h * 64:h * 64 + 64, :]
                    kcT = qk[4][h * 64:h * 64 + 64, :]
                    pscs = []
                    for half in range(2):
                        t = tp + half
                        off = max(0, 128 * (t - 4 * c))
                        psc = psC.tile([128, 512], F32, tag="mk",
                                       name="psc", bufs=2)
                        nc.tensor.matmul(
                            psc[:, off:512],
                            kcT[:, 128 * t:128 * t + 128],
                            qcT[:, 512 * c + off:512 * (c + 1)],
                            start=True, stop=True,
                        )
                        pscs.append(psc)
                    return pscs

                def emit_psc_pair(tp):
                    # both heads' K=64 matmuls adjacent -> disjoint PE row
                    # groups run concurrently
                    out = {0: [], 1: []}
                    for half in range(2):
                        t = tp + half
                        off = max(0, 128 * (t - 4 * c))
                        for h in range(2):
                            qcT = qk[3][h * 64:h * 64 + 64, :]
                            kcT = qk[4][h * 64:h * 64 + 64, :]
                            psc = psC.tile([128, 512], F32, tag="mk",
                                           name="psc", bufs=2)
                            nc.tensor.matmul(
                                psc[:, off:512],
                                kcT[:, 128 * t:128 * t + 128],
                                qcT[:, 512 * c + off:512 * (c + 1)],
                                start=True, stop=True,
                            )
                            out[h].append(psc)
                    return out

                def emit_post(tp, h, psu, pscs):
                    pxp = wk.tile([128, 1024], F16, tag="pexp",
                                  name="pexp", bufs=4)
                    th = wk.tile([128, 1024], F16, tag="th", name="th",
                                 bufs=3)
                    ngs = wk.tile([128, 1024], F16, tag="ngs",
                                  name="ngs", bufs=2)
                    acc = wk.tile([128, 1], F32, tag="acc",
                                  name="acc", bufs=2)
                    if tp + 1 < 4 * c:
                        nc.scalar.activation(th[:], psu[:], AF.Tanh,
                                             scale=0.5)
                        nc.vector._custom_dve(
                            AFFINE_MUL_REDUCE, out=ngs[:], in0=th[:],
                            in1=psu[:], s0=-0.5, s1=-0.5, accum_out=acc[:])
                        for half in range(2):
                            hof = 512 * half
                            nc.vector.tensor_add(
                                ngs[:, hof:hof + 512], pscs[half][:],
                                ngs[:, hof:hof + 512]
                            )
                        nc.scalar.activation(pxp[:], ngs[:], AF.Exp)
                    else:
                        for half in range(2):
                            t = tp + half
                            hof = 512 * half
                            off = max(0, 128 * (t - 4 * c))
                            a, b = hof + off, hof + 512
                            nc.scalar.activation(
                                th[:, a:b], psu[:, a:b], AF.Tanh, scale=0.5)
                            nc.vector._custom_dve(
                                AFFINE_MUL_REDUCE, out=ngs[:, a:b],
                                in0=th[:, a:b], in1=psu[:, a:b],
                                s0=-0.5, s1=-0.5, accum_out=acc[:])
                            nc.vector.tensor_add(
                                ngs[:, a:b], pscs[half][:, off:512],
                                ngs[:, a:b]
                            )
                            nc.scalar.activation(
                                pxp[:, a:b], ngs[:, a:b], AF.Exp)
                            if off > 0:
                                nc.vector.memset(pxp[:, hof:a], 0.0)
                            nc.vector.tensor_mul(
                                pxp[:, a:a + 128], pxp[:, a:a + 128], mD
                            )
                    return pxp

                def emit_pots(tp, h, pxp):
                    for half in range(2):
                        t = tp + half
                        hof = 512 * half
                        nc.tensor.matmul(
                            pots[h][:],
                            vca[h][:, VCW * t:VCW * t + 65],
                            pxp[:, hof:hof + 512],
                            start=(t == 0), stop=(t == tmax),
                        )

                # software pipeline: pots for pair p-1 are emitted between
                # pair p's matmuls so the in-order PE stream never waits on
                # the just-issued ACT/DVE chain
                pending = []
                for tp in range(0, tmax + 1, 2):
                    psus = emit_psu_pair(tp)
                    pscs2 = emit_psc_pair(tp)
                    for h in range(2):
                        if len(pending) >= 2:
                            emit_pots(*pending.pop(0))
                        pxp = emit_post(tp, h, psus[h], pscs2[h])
                        pending.append((tp, h, pxp))
                while pending:
                    emit_pots(*pending.pop(0))

                for h in range(2):
                    potd = wk.tile([1, 512], F32, tag="potd", name="potd",
                                   bufs=1)
                    nc.vector.tensor_copy(potd[:], pots[h][64:65, :])
                    rec = wk.tile([1, 512], F32, tag="rec", name="rec",
                                  bufs=1)
                    nc.vector.reciprocal_approx_fast(out=rec[:], in_=potd[:])
                    recb = wk.tile([64, 512], F32, tag="recb", name="recb",
                                   bufs=1)
                    nc.gpsimd.partition_broadcast(recb[:], rec[:],
                                                  channels=64)
                    nc.vector.tensor_mul(otn[h][:, csl], pots[h][0:64, :],
                                         recb[:])

            def emit_y(c):
                for it in range(4 * c, 4 * c + 4):
                    for dc in range(2):
                        py = psC.tile([128, 512], F32, tag="mk", name="py",
                                      bufs=2)
                        nc.tensor.matmul(
                            py[:],
                            otn[0][:, it * 128:(it + 1) * 128],
                            wo[:, dc * 512:(dc + 1) * 512],
                            start=True, stop=False,
                        )
                        nc.tensor.matmul(
                            py[:],
                            otn[1][:, it * 128:(it + 1) * 128],
                            wo[:, 1024 + dc * 512:1024 + (dc + 1) * 512],
                            start=False, stop=True,
                        )
                        ysb = wk.tile([128, 512], F16, tag="ysb",
                                      name="ysb", bufs=2)
                        copy_any(ysb[:], py[:])
                        nc.sync.dma_start(
                            out=y.ap()[it * 128:(it + 1) * 128,
                                       dc * 512:(dc + 1) * 512],
                            in_=ysb[:],
                        )

            # build pieces keyed by the window whose qk columns complete them
            pieces = {0: [], 1: [], 2: [], 3: []}
            for jt in range(NT):
                pieces[jt // 4].append(("LT", jt, 0))
                pieces[jt // 4].append(("T1", jt, 0))

            # ---- single dense pipeline over 4 token windows ----
            for c in range(4):
                emit_proj(c)
                if c >= 1:
                    emit_y(c - 1)
                for kind, jt, ic in pieces[c]:
                    ensure_tiles(jt)
                    if kind == "LT":
                        emit_lt_row(jt)
                    else:
                        emit_t1_row(jt)
                emit_transposes(c)
                if c >= 1:
                    w_update(c)
                chunks_both(c)
            emit_y(3)

    nc.compile()
    return nc


class _SpmdRunner:
    def __init__(self, nc, n_cores=8):
        import jax
        from jax.sharding import Mesh, PartitionSpec
        from jax.experimental.shard_map import shard_map
        import concourse.mybir as mybir
        from concourse import bass2jax
        from concourse.bass2jax import _bass_exec_p, install_neuronx_cc_hook

        install_neuronx_cc_hook()
        self.jax = jax
        self.nc = nc
        self.n_cores = n_cores
        partition_name = (
            nc.partition_id_tensor.name if nc.partition_id_tensor else None
        )
        in_names, out_names, out_avals = [], [], []
        for alloc in nc.m.functions[0].allocations:
            if not isinstance(alloc, mybir.MemoryLocationSet):
                continue
            name = alloc.memorylocations[0].name
            if alloc.kind == "ExternalInput":
                if name != partition_name:
                    in_names.append(name)
            elif alloc.kind == "ExternalOutput":
                out_names.append(name)
                out_avals.append(
                    jax.core.ShapedArray(
                        tuple(alloc.tensor_shape), mybir.dt.np(alloc.dtype)
                    )
                )
        if nc.dbg_addr is not None:
            assert not nc.dbg_callbacks
            in_names.append(nc.dbg_addr.name)
            self.dbg_name = nc.dbg_addr.name
        else:
            self.dbg_name = None
        self.in_names = list(in_names)
        self.out_names = out_names
        self.out_avals = out_avals

        all_in_names = list(in_names)
        if partition_name is not None:
            all_in_names.append(partition_name)

        def _body(*args):
            operands = list(args)
            if partition_name is not None:
                operands.append(bass2jax.partition_id_tensor())
            outs = _bass_exec_p.bind(
                *operands,
                out_avals=tuple(out_avals),
                in_names=tuple(all_in_names),
                out_names=tuple(out_names),
                lowering_input_output_aliases=(),
                sim_require_finite=True,
                sim_require_nnan=True,
                nc=nc,
            )
            return tuple(outs)

        devices = jax.devices()[:n_cores]
        assert len(devices) == n_cores
        self.mesh = Mesh(np.asarray(devices), ("core",))
        in_specs = (PartitionSpec("core"),) * len(in_names)
        out_specs = (PartitionSpec("core"),) * len(out_names)
        self.fn = jax.jit(
            shard_map(
                _body,
                mesh=self.mesh,
                in_specs=in_specs,
                out_specs=out_specs,
                check_rep=False,
            ),
            keep_unused=True,
        )
        self.in_sharding = jax.sharding.NamedSharding(
            self.mesh, PartitionSpec("core")
        )

    def put_inputs(self, in_maps):
        assert len(in_maps) == self.n_cores
        if self.dbg_name is not None:
            in_maps = [
                {**m, self.dbg_name: np.zeros((1, 2), np.uint32)} for m in in_maps
            ]
        args = []
        for name in self.in_names:
            cat = np.concatenate(
                [np.asarray(in_maps[c][name]) for c in range(self.n_cores)],
                axis=0,
            )
            args.append(self.jax.device_put(cat, self.in_sharding))
        return args

    def run(self, dev_args):
        outs = self.fn(*dev_args)
        self.jax.block_until_ready(outs)
        return outs

    def outputs_to_host(self, outs):
        res = []
        for c in range(self.n_cores):
            d = {}
            for i, name in enumerate(self.out_names):
                d[name] = np.asarray(outs[i]).reshape(
                    self.n_cores, *self.out_avals[i].shape
                )[c]
            res.append(d)
        return res

    def __call__(self, in_maps):
        return self.outputs_to_host(self.run(self.put_inputs(in_maps)))


def _get_state():
    if "runner" not in _STATE:
        nc = _build_nc()
        _STATE["nc"] = nc
        _STATE["runner"] = _SpmdRunner(nc, 8)
    return _STATE


def make_in_maps(x, W_qkv, W_out):
    x = np.asarray(x, dtype=np.float32)
    W_qkv = np.asarray(W_qkv, dtype=np.float32)
    W_out = np.asarray(W_out, dtype=np.float32)

    ar = np.arange(128)
    mD = (ar[None, :] >= ar[:, None]).astype(np.float16)
    mL = (0.5 * (ar[None, :] < ar[:, None])).astype(np.float16)
    idn = np.eye(128, dtype=np.float16)
    onescol = np.ones((128, 8), np.float16)

    # x pack: xTp[p, nch*4096 + dtile*512 + cc] = x[b][nch*512+cc, dtile*128+p]
    xTp = []
    for b in range(B):
        xT = x[b].T                                   # [D, N]
        xp = xT.reshape(8, 128, 4, 512).transpose(1, 2, 0, 3).reshape(128, -1)
        xTp.append(np.ascontiguousarray(xp.astype(np.float16)))

    in_maps = []
    for c in range(8):
        b, p = c // 4, c % 4
        rows = []
        for qkv in range(6):
            for hl in range(2):
                h = 2 * p + hl
                blk = W_qkv[qkv * 512 + h * 64:qkv * 512 + h * 64 + 64, :]
                if qkv in (0, 3):
                    blk = blk * SCALE
                rows.append(blk)
        wq = np.concatenate(rows, axis=0)             # [768, D]
        # pack: wqp[p, dtile*768 + cc] = wq[cc, dtile*128+p]
        wqT = wq.T                                    # [D, 768]
        wqpk = wqT.reshape(8, 128, 768).transpose(1, 0, 2).reshape(128, -1)
        wqpk = np.ascontiguousarray(wqpk.astype(np.float16))
        wo0T = W_out[:, 128 * p:128 * p + 64].T.astype(np.float16)  # [64, D]
        wo1T = W_out[:, 128 * p + 64:128 * p + 128].T.astype(np.float16)
        woP = np.zeros((128, 2048), np.float16)
        woP[0:64, 0:1024] = wo0T
        woP[0:64, 1024:2048] = wo1T
        cst = np.concatenate([mD, mL, idn, woP, onescol], axis=1)
        assert cst.shape == (128, C_W)
        in_maps.append({
            "xTp": xTp[b], "wqp": wqpk, "cstd": np.ascontiguousarray(cst),
        })
    return in_maps


def kernel(x, W_qkv, W_out):
    st = _get_state()
    in_maps = make_in_maps(x, W_qkv, W_out)
    res = st["runner"](in_maps)
    out = np.zeros((B, N, D), np.float32)
    for c in range(8):
        out[c // 4] += res[c]["y"].astype(np.float32)
    return out


if __name__ == "__main__":
    rng = np.random.default_rng(0)
    x = rng.standard_normal((B, N, D)).astype(np.float32)
    W_qkv = (rng.standard_normal((6 * 512, D)) * 0.02).astype(np.float32)
    W_out = (rng.standard_normal((D, 512)) * 0.02).astype(np.float32)
    y = kernel(x, W_qkv, W_out)
    print("kernel ran, out shape", y.shape, "finite:", np.isfinite(y).all())


# revision 22
# speedup vs baseline: 1.0972x; 1.0972x over previous
"""Sparse-attention ("Castle") Trainium2 kernel, 8-core SPMD.

Sharding: core c handles batch b = c // 4 and head pair p = c % 4
(heads 2p, 2p+1). All fp16 operands (scale pre-folded host-side).

Single dense 4-window pipeline per core: each 512-token window
interleaves QKV projection, lookahead^T / term1^T builds, the
block-sparse Su contraction chunk, softmax, and the output projection
so the tensor engine always has independent matmuls in flight (HAM
stays at full clock as much as possible).

Su interior is collapsed via a cumulative rank-64 matrix W_c^T =
sum_{j<512c} sigma_kj vu_j^T (one K=64 matmul per k-tile instead of
up to 12 K=128 matmuls); only the 4 boundary j-tiles per chunk use
the explicit term1^T rows, so term1^T shrinks to one narrow row per
j-tile. Post-processing uses only the {tanh, exp, copy} activation
table (no table thrash): -silu(Su) comes from one AFFINE_MUL_REDUCE
custom-DVE op, sigma fixups run on gpsimd, and softmax normalization
uses reciprocal_approx_fast + gpsimd partition broadcast. The pots
(attn @ vc) matmuls are software-pipelined two pair-heads behind
their exp producers. Host sums the 4 fp16 partials per batch.
"""
import sys
sys.path.insert(0, "/opt/trn_rl_repo")
import numpy as np

B, N, D = 2, 2048, 1024
HEADS, DH = 8, 64
NT = N // 128          # 16 row tiles
SCALE = DH ** -0.5
VCW = 66               # stride of one vcA block (64 vc cols + 1 ones + pad)

# const pack layout (columns of cst tile)
C_MD = 0        # [128,128] diag mask: col >= row
C_ML = 128      # [128,128] strictly-lower mask: col < row
C_ID = 256      # [128,128] identity
C_WO = 384      # [64,2048] W_out slices, cols h*1024+d (partitions 0-63)
C_ON = 2432     # [128,8] ones
C_W = 2440

_STATE = {}


def _build_nc():
    import concourse.bacc as bacc
    import concourse.mybir as mybir
    from concourse import tile
    from concourse.dve_ops import AFFINE_MUL_REDUCE

    F32 = mybir.dt.float32
    F16 = mybir.dt.float16
    AF = mybir.ActivationFunctionType

    nc = bacc.Bacc("TRN2", target_bir_lowering=False, debug=False)

    xTp = nc.dram_tensor("xTp", [128, 4 * 4096], F16, kind="ExternalInput")
    wqp = nc.dram_tensor("wqp", [128, 8 * 768], F16, kind="ExternalInput")
    cstd = nc.dram_tensor("cstd", [128, C_W], F16, kind="ExternalInput")
    y = nc.dram_tensor("y", [N, D], F16, kind="ExternalOutput")

    with tile.TileContext(nc) as tc:
        with tc.tile_pool(name="const", bufs=1) as cstp, \
             tc.tile_pool(name="qkv", bufs=1) as qkvp, \
             tc.tile_pool(name="vca", bufs=1) as vcap, \
             tc.tile_pool(name="otn", bufs=1) as otnp, \
             tc.tile_pool(name="t1p", bufs=1) as t1p, \
             tc.tile_pool(name="ltp", bufs=1) as ltp, \
             tc.tile_pool(name="wk", bufs=3) as wk, \
             tc.tile_pool(name="xw", bufs=1) as xw, \
             tc.tile_pool(name="psC", bufs=2, space="PSUM") as psC:

            cst = cstp.tile([128, C_W], F16, tag="cst")
            mD = cst[:, C_MD:C_MD + 128]
            mL = cst[:, C_ML:C_ML + 128]
            idn = cst[:, C_ID:C_ID + 128]
            wo = cst[0:64, C_WO:C_WO + 2048]
            ones = cst[:, C_ON:C_ON + 1]

            qk = [qkvp.tile([128, N], F16, tag=f"qk{j}", name=f"qk{j}")
                  for j in range(5)]
            vca = [vcap.tile([128, VCW * NT], F16, tag=f"vca{h}",
                             name=f"vca{h}") for h in range(2)]
            vua = [vcap.tile([128, 64 * NT], F16, tag=f"vua{h}",
                             name=f"vua{h}") for h in range(2)]
            otn = [otnp.tile([64, N], F16, tag=f"otn{h}", name=f"otn{h}")
                   for h in range(2)]
            # cumulative interior weights W_c^T, rows h*64+dh, cols k
            wt = otnp.tile([128, N], F16, tag="wt", name="wt")
            nc.vector.memset(wt[:], 0.0)

            vcT = xw.tile([128, N], F16, tag="vcT", name="vcT")
            wq = xw.tile([128, 8 * 768], F16, tag="wq", name="wq")
            nc.sync.dma_start(out=wq[:], in_=wqp.ap())
            xt = []
            for nch in range(4):
                xti = xw.tile([128, 4096], F16, tag="xti", name=f"x{nch}",
                              bufs=2)
                nc.sync.dma_start(
                    out=xti[:],
                    in_=xTp.ap()[:, nch * 4096:(nch + 1) * 4096],
                )
                xt.append(xti)
            nc.sync.dma_start(out=cst[:], in_=cstd.ap())

            t1t = {}
            ltt = {}
            tg = [0]

            def copy_any(dst, src):
                if tg[0] % 2 == 0:
                    nc.vector.tensor_copy(dst, src)
                else:
                    nc.scalar.activation(dst, src, AF.Copy)
                tg[0] += 1

            def ensure_tiles(jt):
                if (0, jt) in t1t:
                    return
                r = jt % 4
                for h in range(2):
                    t1t[(h, jt)] = t1p.tile(
                        [128, 512 - 128 * r], F16, tag=f"t1r{r}_{h}",
                        name=f"t1_{h}_{jt}", bufs=2)
                    ltt[(h, jt)] = ltp.tile(
                        [128, 128 * (jt + 1)], F16, tag=f"lt_{h}_{jt}",
                        name=f"lt_{h}_{jt}")

            def emit_proj(nch):
                nsl = slice(nch * 512, nch * 512 + 512)
                for j in range(6):
                    pp = psC.tile([128, 512], F32, tag="mk", name="pp", bufs=2)
                    for dtile in range(8):
                        nc.tensor.matmul(
                            pp[:],
                            wq[:, dtile * 768 + j * 128:
                               dtile * 768 + (j + 1) * 128],
                            xt[nch][:, dtile * 512:(dtile + 1) * 512],
                            start=(dtile == 0),
                            stop=(dtile == 7),
                        )
                    dst = vcT if j == 5 else qk[j]
                    nc.scalar.activation(dst[:, nsl], pp[:], AF.Copy)

            def emit_transposes(nch):
                for kt in range(4 * nch, 4 * nch + 4):
                    pt = psC.tile([128, 128], F16, tag="mk", name="pt", bufs=2)
                    nc.tensor.transpose(
                        pt[:], vcT[:, kt * 128:(kt + 1) * 128], idn)
                    for h in range(2):
                        nc.vector.tensor_copy(
                            vca[h][:, kt * VCW:kt * VCW + 64],
                            pt[:, h * 64:(h + 1) * 64],
                        )
                        nc.vector.tensor_copy(
                            vca[h][:, kt * VCW + 64:kt * VCW + 65], ones
                        )
                    pu = psC.tile([128, 128], F16, tag="mk", name="pu", bufs=2)
                    nc.tensor.transpose(
                        pu[:], qk[2][:, kt * 128:(kt + 1) * 128], idn)
                    for h in range(2):
                        nc.vector.tensor_copy(
                            vua[h][:, kt * 64:(kt + 1) * 64],
                            pu[:, h * 64:(h + 1) * 64],
                        )

            def emit_t1_row(jt):
                i0 = 128 * jt
                w = 512 - 128 * (jt % 4)
                pss = []
                for h in range(2):
                    vuT = qk[2][h * 64:h * 64 + 64, :]
                    qcT = qk[3][h * 64:h * 64 + 64, :]
                    ps = psC.tile([128, 512], F32, tag="mk",
                                  name="mkp", bufs=2)
                    nc.tensor.matmul(
                        ps[:, 0:w],
                        vuT[:, jt * 128:(jt + 1) * 128],
                        qcT[:, i0:i0 + w],
                        start=True, stop=True,
                    )
                    pss.append(ps)
                for h in range(2):
                    t1 = t1t[(h, jt)]
                    ps = pss[h]
                    nc.vector.tensor_mul(t1[:, 0:128], ps[:, 0:128], mD)
                    if w > 128:
                        copy_any(t1[:, 128:w], ps[:, 128:w])

            def emit_lt_row(jt):
                Lk = 128 * (jt + 1)
                for k0 in range(0, Lk, 512):
                    w = min(512, Lk - k0)
                    pss = []
                    for h in range(2):
                        quT = qk[0][h * 64:h * 64 + 64, :]
                        kuT = qk[1][h * 64:h * 64 + 64, :]
                        ps = psC.tile([128, 512], F32, tag="mk",
                                      name="mkp2", bufs=2)
                        nc.tensor.matmul(
                            ps[:, 0:w],
                            kuT[:, jt * 128:(jt + 1) * 128],
                            quT[:, k0:k0 + w],
                            start=True, stop=True,
                        )
                        pss.append(ps)
                    last = (k0 + w == Lk)
                    for h in range(2):
                        lt = ltt[(h, jt)]
                        thL = wk.tile([128, 512], F16, tag="thL",
                                      name="thL", bufs=2)
                        nc.scalar.activation(thL[:, 0:w], pss[h][:, 0:w],
                                             AF.Tanh, scale=0.5)
                        wa = w - 128 if last else w
                        if wa > 0:
                            nc.gpsimd.tensor_scalar(
                                lt[:, k0:k0 + wa], thL[:, 0:wa], 0.5, 0.5,
                                op0=mybir.AluOpType.mult,
                                op1=mybir.AluOpType.add,
                            )
                        if last:
                            # lt_diag = (th + 1) * (0.5*mask)
                            nc.vector.scalar_tensor_tensor(
                                lt[:, Lk - 128:Lk], thL[:, w - 128:w], 1.0, mL,
                                op0=mybir.AluOpType.add,
                                op1=mybir.AluOpType.mult,
                            )

            def w_update(c):
                # add j-window [512(c-1), 512c) into W^T for k-tiles < 4c
                for h in range(2):
                    for kt in range(0, 4 * c):
                        jts = [jt for jt in range(4 * (c - 1), 4 * c)
                               if jt >= kt]
                        if not jts:
                            continue
                        pd = psC.tile([128, 128], F32, tag="mk",
                                      name="pd", bufs=2)
                        for ji, jt in enumerate(jts):
                            nc.tensor.matmul(
                                pd[h * 64:h * 64 + 64, :],
                                vua[h][:, jt * 64:(jt + 1) * 64],
                                ltt[(h, jt)][:, kt * 128:(kt + 1) * 128],
                                start=(ji == 0), stop=(ji == len(jts) - 1),
                            )
                        nc.vector.tensor_add(
                            wt[h * 64:h * 64 + 64, kt * 128:(kt + 1) * 128],
                            wt[h * 64:h * 64 + 64, kt * 128:(kt + 1) * 128],
                            pd[h * 64:h * 64 + 64, :],
                        )

            def chunks_both(c):
                csl = slice(512 * c, 512 * c + 512)
                tmax = 4 * c + 3
                pots = {}
                for h in range(2):
                    pots[h] = psC.tile([65, 512], F32, tag=f"ot{h}",
                                       name=f"pot{h}", bufs=1)

                def emit_psu(tp, h):
                    qcT = qk[3][h * 64:h * 64 + 64, :]
                    psu = psC.tile([128, 1024], F32, tag="su",
                                   name="psu", bufs=2)
                    for half in range(2):
                        t = tp + half
                        hof = 512 * half
                        started = False
                        if t < 4 * c:
                            nc.tensor.matmul(
                                psu[:, hof:hof + 512],
                                wt[h * 64:h * 64 + 64, t * 128:(t + 1) * 128],
                                qcT[:, csl],
                                start=True, stop=False,
                            )
                            started = True
                        for jt in range(max(t, 4 * c), tmax + 1):
                            s_loc = 512 * c - 128 * jt
                            if s_loc >= 0:
                                nc.tensor.matmul(
                                    psu[:, hof:hof + 512],
                                    ltt[(h, jt)][:, 128 * t:128 * t + 128],
                                    t1t[(h, jt)][:, s_loc:s_loc + 512],
                                    start=not started, stop=(jt == tmax),
                                )
                            else:
                                nc.tensor.matmul(
                                    psu[:, hof - s_loc:hof + 512],
                                    ltt[(h, jt)][:, 128 * t:128 * t + 128],
                                    t1t[(h, jt)][:, 0:512 + s_loc],
                                    start=not started, stop=(jt == tmax),
                                )
                            started = True
                    return psu

                def emit_psc(tp, h):
                    qcT = qk[3][h * 64:h * 64 + 64, :]
                    kcT = qk[4][h * 64:h * 64 + 64, :]
                    pscs = []
                    for half in range(2):
                        t = tp + half
                        off = max(0, 128 * (t - 4 * c))
                        psc = psC.tile([128, 512], F32, tag="mk",
                                       name="psc", bufs=2)
                        nc.tensor.matmul(
                            psc[:, off:512],
                            kcT[:, 128 * t:128 * t + 128],
                            qcT[:, 512 * c + off:512 * (c + 1)],
                            start=True, stop=True,
                        )
                        pscs.append(psc)
                    return pscs

                def emit_psc_pair(tp):
                    # both heads' K=64 matmuls adjacent -> disjoint PE row
                    # groups run concurrently
                    out = {0: [], 1: []}
                    for half in range(2):
                        t = tp + half
                        off = max(0, 128 * (t - 4 * c))
                        for h in range(2):
                            qcT = qk[3][h * 64:h * 64 + 64, :]
                            kcT = qk[4][h * 64:h * 64 + 64, :]
                            psc = psC.tile([128, 512], F32, tag="mk",
                                           name="psc", bufs=2)
                            nc.tensor.matmul(
                                psc[:, off:512],
                                kcT[:, 128 * t:128 * t + 128],
                                qcT[:, 512 * c + off:512 * (c + 1)],
                                start=True, stop=True,
                            )
                            out[h].append(psc)
                    return out

                def emit_post(tp, h, psu, pscs):
                    pxp = wk.tile([128, 1024], F16, tag="pexp",
                                  name="pexp", bufs=4)
                    th = wk.tile([128, 1024], F16, tag="th", name="th",
                                 bufs=3)
                    ngs = wk.tile([128, 1024], F16, tag="ngs",
                                  name="ngs", bufs=2)
                    acc = wk.tile([128, 1], F32, tag="acc",
                                  name="acc", bufs=2)
                    if tp + 1 < 4 * c:
                        nc.scalar.activation(th[:], psu[:], AF.Tanh,
                                             scale=0.5)
                        nc.vector._custom_dve(
                            AFFINE_MUL_REDUCE, out=ngs[:], in0=th[:],
                            in1=psu[:], s0=-0.5, s1=-0.5, accum_out=acc[:])
                        for half in range(2):
                            hof = 512 * half
                            nc.vector.tensor_add(
                                ngs[:, hof:hof + 512], pscs[half][:],
                                ngs[:, hof:hof + 512]
                            )
                        nc.scalar.activation(pxp[:], ngs[:], AF.Exp)
                    else:
                        for half in range(2):
                            t = tp + half
                            hof = 512 * half
                            off = max(0, 128 * (t - 4 * c))
                            a, b = hof + off, hof + 512
                            nc.scalar.activation(
                                th[:, a:b], psu[:, a:b], AF.Tanh, scale=0.5)
                            nc.vector._custom_dve(
                                AFFINE_MUL_REDUCE, out=ngs[:, a:b],
                                in0=th[:, a:b], in1=psu[:, a:b],
                                s0=-0.5, s1=-0.5, accum_out=acc[:])
                            nc.vector.tensor_add(
                                ngs[:, a:b], pscs[half][:, off:512],
                                ngs[:, a:b]
                            )
                            nc.scalar.activation(
                                pxp[:, a:b], ngs[:, a:b], AF.Exp)
                            if off > 0:
                                nc.vector.memset(pxp[:, hof:a], 0.0)
                            nc.vector.tensor_mul(
                                pxp[:, a:a + 128], pxp[:, a:a + 128], mD
                            )
                    return pxp

                def emit_pots(tp, h, pxp):
                    for half in range(2):
                        t = tp + half
                        hof = 512 * half
                        nc.tensor.matmul(
                            pots[h][:],
                            vca[h][:, VCW * t:VCW * t + 65],
                            pxp[:, hof:hof + 512],
                            start=(t == 0), stop=(t == tmax),
                        )

                # software pipeline: pots for pair p-1 are emitted between
                # pair p's matmuls so the in-order PE stream never waits on
                # the just-issued ACT/DVE chain
                pending = []
                for tp in range(0, tmax + 1, 2):
                    psus = [emit_psu(tp, h) for h in range(2)]
                    pscs2 = emit_psc_pair(tp)
                    for h in range(2):
                        if len(pending) >= 2:
                            emit_pots(*pending.pop(0))
                        pxp = emit_post(tp, h, psus[h], pscs2[h])
                        pending.append((tp, h, pxp))
                while pending:
                    emit_pots(*pending.pop(0))

                for h in range(2):
                    potd = wk.tile([1, 512], F32, tag="potd", name="potd",
                                   bufs=1)
                    nc.vector.tensor_copy(potd[:], pots[h][64:65, :])
                    rec = wk.tile([1, 512], F32, tag="rec", name="rec",
                                  bufs=1)
                    nc.vector.reciprocal_approx_fast(out=rec[:], in_=potd[:])
                    recb = wk.tile([64, 512], F32, tag="recb", name="recb",
                                   bufs=1)
                    nc.gpsimd.partition_broadcast(recb[:], rec[:],
                                                  channels=64)
                    nc.vector.tensor_mul(otn[h][:, csl], pots[h][0:64, :],
                                         recb[:])

            def emit_y(c):
                for it in range(4 * c, 4 * c + 4):
                    for dc in range(2):
                        py = psC.tile([128, 512], F32, tag="mk", name="py",
                                      bufs=2)
                        nc.tensor.matmul(
                            py[:],
                            otn[0][:, it * 128:(it + 1) * 128],
                            wo[:, dc * 512:(dc + 1) * 512],
                            start=True, stop=False,
                        )
                        nc.tensor.matmul(
                            py[:],
                            otn[1][:, it * 128:(it + 1) * 128],
                            wo[:, 1024 + dc * 512:1024 + (dc + 1) * 512],
                            start=False, stop=True,
                        )
                        ysb = wk.tile([128, 512], F16, tag="ysb",
                                      name="ysb", bufs=2)
                        copy_any(ysb[:], py[:])
                        nc.sync.dma_start(
                            out=y.ap()[it * 128:(it + 1) * 128,
                                       dc * 512:(dc + 1) * 512],
                            in_=ysb[:],
                        )

            # build pieces keyed by the window whose qk columns complete them
            pieces = {0: [], 1: [], 2: [], 3: []}
            for jt in range(NT):
                pieces[jt // 4].append(("LT", jt, 0))
                pieces[jt // 4].append(("T1", jt, 0))

            # ---- single dense pipeline over 4 token windows ----
            for c in range(4):
                emit_proj(c)
                if c >= 1:
                    emit_y(c - 1)
                for kind, jt, ic in pieces[c]:
                    ensure_tiles(jt)
                    if kind == "LT":
                        emit_lt_row(jt)
                    else:
                        emit_t1_row(jt)
                emit_transposes(c)
                if c >= 1:
                    w_update(c)
                chunks_both(c)
            emit_y(3)

    nc.compile()
    return nc


class _SpmdRunner:
    def __init__(self, nc, n_cores=8):
        import jax
        from jax.sharding import Mesh, PartitionSpec
        from jax.experimental.shard_map import shard_map
        import concourse.mybir as mybir
        from concourse import bass2jax
        from concourse.bass2jax import _bass_exec_p, install_neuronx_cc_hook

        install_neuronx_cc_hook()
        self.jax = jax
        self.nc = nc
        self.n_cores = n_cores
        partition_name = (
            nc.partition_id_tensor.name if nc.partition_id_tensor else None
        )
        in_names, out_names, out_avals = [], [], []
        for alloc in nc.m.functions[0].allocations:
            if not isinstance(alloc, mybir.MemoryLocationSet):
                continue
            name = alloc.memorylocations[0].name
            if alloc.kind == "ExternalInput":
                if name != partition_name:
                    in_names.append(name)
            elif alloc.kind == "ExternalOutput":
                out_names.append(name)
                out_avals.append(
                    jax.core.ShapedArray(
                        tuple(alloc.tensor_shape), mybir.dt.np(alloc.dtype)
                    )
                )
        if nc.dbg_addr is not None:
            assert not nc.dbg_callbacks
            in_names.append(nc.dbg_addr.name)
            self.dbg_name = nc.dbg_addr.name
        else:
            self.dbg_name = None
        self.in_names = list(in_names)
        self.out_names = out_names
        self.out_avals = out_avals

        all_in_names = list(in_names)
        if partition_name is not None:
            all_in_names.append(partition_name)

        def _body(*args):
            operands = list(args)
            if partition_name is not None:
                operands.append(bass2jax.partition_id_tensor())
            outs = _bass_exec_p.bind(
                *operands,
                out_avals=tuple(out_avals),
                in_names=tuple(all_in_names),
                out_names=tuple(out_names),
                lowering_input_output_aliases=(),
                sim_require_finite=True,
                sim_require_nnan=True,
                nc=nc,
            )
            return tuple(outs)

        devices = jax.devices()[:n_cores]
        assert len(devices) == n_cores
        self.mesh = Mesh(np.asarray(devices), ("core",))
        in_specs = (PartitionSpec("core"),) * len(in_names)
        out_specs = (PartitionSpec("core"),) * len(out_names)
        self.fn = jax.jit(
            shard_map(
                _body,
                mesh=self.mesh,
                in_specs=in_specs,
                out_specs=out_specs,
                check_rep=False,
            ),
            keep_unused=True,
        )
        self.in_sharding = jax.sharding.NamedSharding(
            self.mesh, PartitionSpec("core")
        )

    def put_inputs(self, in_maps):
        assert len(in_maps) == self.n_cores
        if self.dbg_name is not None:
            in_maps = [
                {**m, self.dbg_name: np.zeros((1, 2), np.uint32)} for m in in_maps
            ]
        args = []
        for name in self.in_names:
            cat = np.concatenate(
                [np.asarray(in_maps[c][name]) for c in range(self.n_cores)],
                axis=0,
            )
            args.append(self.jax.device_put(cat, self.in_sharding))
        return args

    def run(self, dev_args):
        outs = self.fn(*dev_args)
        self.jax.block_until_ready(outs)
        return outs

    def outputs_to_host(self, outs):
        res = []
        for c in range(self.n_cores):
            d = {}
            for i, name in enumerate(self.out_names):
                d[name] = np.asarray(outs[i]).reshape(
                    self.n_cores, *self.out_avals[i].shape
                )[c]
            res.append(d)
        return res

    def __call__(self, in_maps):
        return self.outputs_to_host(self.run(self.put_inputs(in_maps)))


def _get_state():
    if "runner" not in _STATE:
        nc = _build_nc()
        _STATE["nc"] = nc
        _STATE["runner"] = _SpmdRunner(nc, 8)
    return _STATE


def make_in_maps(x, W_qkv, W_out):
    x = np.asarray(x, dtype=np.float32)
    W_qkv = np.asarray(W_qkv, dtype=np.float32)
    W_out = np.asarray(W_out, dtype=np.float32)

    ar = np.arange(128)
    mD = (ar[None, :] >= ar[:, None]).astype(np.float16)
    mL = (0.5 * (ar[None, :] < ar[:, None])).astype(np.float16)
    idn = np.eye(128, dtype=np.float16)
    onescol = np.ones((128, 8), np.float16)

    # x pack: xTp[p, nch*4096 + dtile*512 + cc] = x[b][nch*512+cc, dtile*128+p]
    xTp = []
    for b in range(B):
        xT = x[b].T                                   # [D, N]
        xp = xT.reshape(8, 128, 4, 512).transpose(1, 2, 0, 3).reshape(128, -1)
        xTp.append(np.ascontiguousarray(xp.astype(np.float16)))

    in_maps = []
    for c in range(8):
        b, p = c // 4, c % 4
        rows = []
        for qkv in range(6):
            for hl in range(2):
                h = 2 * p + hl
                blk = W_qkv[qkv * 512 + h * 64:qkv * 512 + h * 64 + 64, :]
                if qkv in (0, 3):
                    blk = blk * SCALE
                rows.append(blk)
        wq = np.concatenate(rows, axis=0)             # [768, D]
        # pack: wqp[p, dtile*768 + cc] = wq[cc, dtile*128+p]
        wqT = wq.T                                    # [D, 768]
        wqpk = wqT.reshape(8, 128, 768).transpose(1, 0, 2).reshape(128, -1)
        wqpk = np.ascontiguousarray(wqpk.astype(np.float16))
        wo0T = W_out[:, 128 * p:128 * p + 64].T.astype(np.float16)  # [64, D]
        wo1T = W_out[:, 128 * p + 64:128 * p + 128].T.astype(np.float16)
        woP = np.zeros((128, 2048), np.float16)
        woP[0:64, 0:1024] = wo0T
        woP[0:64, 1024:2048] = wo1T
        cst = np.concatenate([mD, mL, idn, woP, onescol], axis=1)
        assert cst.shape == (128, C_W)
        in_maps.append({
            "xTp": xTp[b], "wqp": wqpk, "cstd": np.ascontiguousarray(cst),
        })
    return in_maps


def kernel(x, W_qkv, W_out):
    st = _get_state()
    in_maps = make_in_maps(x, W_qkv, W_out)
    res = st["runner"](in_maps)
    out = np.zeros((B, N, D), np.float32)
    for c in range(8):
        out[c // 4] += res[c]["y"].astype(np.float32)
    return out


if __name__ == "__main__":
    rng = np.random.default_rng(0)
    x = rng.standard_normal((B, N, D)).astype(np.float32)
    W_qkv = (rng.standard_normal((6 * 512, D)) * 0.02).astype(np.float32)
    W_out = (rng.standard_normal((D, 512)) * 0.02).astype(np.float32)
    y = kernel(x, W_qkv, W_out)
    print("kernel ran, out shape", y.shape, "finite:", np.isfinite(y).all())
